# revision 15
# baseline (speedup 1.0000x reference)
"""Self-contained Trainium2 Bass kernel for nn_AttentionBlock (B2 H64 W64 C512).

Module: GroupNorm(32 groups) -> 1x1 conv q,k,v -> full [N,N] softmax attention
        -> 1x1 proj -> residual.

Sharding: 8 cores = 2 batches x 4 query-blocks (1024 rows each).  Each core
gets its batch's full image transposed to [C, N] bf16 with the token axis
rotated so its own query block is always columns 0..1023 (attention is
permutation-invariant over keys, so the rotation needs no undo on the key
side).  GroupNorm coefficients land on the partition axis, K/V are computed
for all 4096 tokens (replicated within the 4-core batch group), attention
keeps keys on the partition axis throughout (logits here are tiny, |s|<2,
so softmax needs no max subtraction), and the fp32 residual is added from
an untransposed per-core block.
"""

import numpy as np
import ml_dtypes

B, H, W, C = 2, 64, 64, 512
N = H * W            # 4096 tokens per batch
GROUPS, GS = 32, 16
EPS = 1e-5
NCORES = 8
RPB = 4              # query row-blocks per batch
QB = N // RPB        # 1024 queries per core
CCH = C // 128       # 4 channel chunks
TT = N // 512        # 8 token tiles of 512
TC = N // 128        # 32 token chunks of 128
PANELS = QB // 512   # query panels of 512 per core
SCALE = float(C) ** -0.5
INV_CNT = 1.0 / (N * GS)

_BF16 = ml_dtypes.bfloat16
_BUILT = {}


def _emit(nc, tc, ap, loop_ab=0, loop_c=0, stage=3, tune=None):
    tune = tune or {}
    import concourse.bass as bass
    from concourse import mybir
    from contextlib import nullcontext

    dt = mybir.dt
    AF = mybir.ActivationFunctionType
    ALU = mybir.AluOpType
    AX = mybir.AxisListType
    ts = bass.ts

    with tc.tile_pool(name="persist", bufs=1) as P:
        # ---- persistent SBUF tiles ---------------------------------------
        kt = [P.tile([128, N], dt.bfloat16, tag=f"kt{i}", name=f"kt{i}") for i in range(CCH)]
        qt = [P.tile([128, QB], dt.bfloat16, tag=f"qt{i}", name=f"qt{i}") for i in range(CCH)]
        v_sb = P.tile([128, TC * 512], dt.bfloat16, tag="v")  # [tok%128,(tc,c)]
        w_sb = {}
        for wname in ("wq", "wk", "wv", "wp"):
            w_sb[wname] = P.tile([128, CCH * 512], dt.bfloat16, tag=wname, name=wname + "_sb")
            nc.sync.dma_start(w_sb[wname][:],
                              ap[wname].rearrange("(cc p) m -> p cc m", p=128))
        bq_sb = P.tile([128, CCH], dt.float32, tag="bq")
        bk_sb = P.tile([128, CCH], dt.float32, tag="bk")
        nc.sync.dma_start(bq_sb[:], ap["bq_t"].rearrange("(cc p) o -> p cc o", p=128))
        nc.sync.dma_start(bk_sb[:], ap["bk_t"].rearrange("(cc p) o -> p cc o", p=128))
        bvb_sb = P.tile([128, 512], dt.float32, tag="bvb")
        bpb_sb = P.tile([128, 512], dt.float32, tag="bpb")
        nc.sync.dma_start(bvb_sb[:], ap["bv_b"][:])
        nc.sync.dma_start(bpb_sb[:], ap["bp_b"][:])
        gam_sb = P.tile([128, CCH], dt.float32, tag="gam")
        bet_sb = P.tile([128, CCH], dt.float32, tag="bet")
        nc.sync.dma_start(gam_sb[:], ap["gam_t"].rearrange("(cc p) o -> p cc o", p=128))
        nc.sync.dma_start(bet_sb[:], ap["bet_t"].rearrange("(cc p) o -> p cc o", p=128))
        g_sb = P.tile([128, 8], dt.float32, tag="g")
        gt_sb = P.tile([8, 128], dt.float32, tag="gt")
        nc.sync.dma_start(g_sb[:], ap["gmat"][:])
        nc.sync.dma_start(gt_sb[:], ap["gmat_t"][:])
        ones_bf = P.tile([128, 1], dt.bfloat16, tag="ones_bf")
        nc.vector.memset(ones_bf[:], 1.0)
        ones1_f = P.tile([1, 128], dt.float32, tag="ones1_f")
        nc.vector.memset(ones1_f[:], 1.0)
        xpb = P.tile([128, RPB * 2 * 512], dt.float32, tag="xpb")
        nc.sync.dma_start(xpb[:], ap["xblk"].rearrange("(qc p) c -> p qc c", p=128))
        for gqc in range(2 * RPB):   # fold proj bias into the residual block
            nc.vector.tensor_add(xpb[:, ts(gqc, 512)], xpb[:, ts(gqc, 512)],
                                 bpb_sb[:])
        st = P.tile([128, 2 * CCH], dt.float32, tag="st")
        a_t = P.tile([128, CCH], dt.float32, tag="a_t")
        b_t = P.tile([128, CCH], dt.float32, tag="b_t")

        # =================================================================
        # Phases A+B: GroupNorm stats + normalize -> ht, then QKV.
        # =================================================================
        with (
            tc.tile_pool(name="pin", bufs=1) as pin,
            tc.tile_pool(name="small", bufs=4) as small,
        ):
            xt = [pin.tile([128, N], dt.bfloat16, tag=f"xt{i}", name=f"xt{i}") for i in range(CCH)]
            sq_scr = pin.tile([128, N], dt.bfloat16, tag="sq_scr")
            eps8 = pin.tile([8, 1], dt.float32, tag="eps8")
            nc.vector.memset(eps8[:], EPS)
            b_bf = pin.tile([128, CCH], dt.bfloat16, tag="b_bf")
            bk2 = pin.tile([128, CCH], dt.float32, tag="bk2")
            bq2 = pin.tile([128, CCH], dt.float32, tag="bq2")
            bvb2 = pin.tile([128, 512], dt.float32, tag="bvb2")
            for ci in range(CCH):
                nc.sync.dma_start(xt[ci][:], ap["xt"][ci * 128:(ci + 1) * 128, :])

            with tc.tile_pool(name="pstat", bufs=1, space="PSUM") as pstat, \
                    tc.tile_pool(name="pk", bufs=4, space="PSUM") as pk, \
                    (tc.For_i(0, loop_ab, 1) if loop_ab else nullcontext()):
                for ci in range(CCH):
                    nc.vector.reduce_sum(st[:, 2 * ci:2 * ci + 1], xt[ci][:],
                                         axis=AX.X)
                    nc.scalar.activation(sq_scr[:], xt[ci][:], AF.Square,
                                         accum_out=st[:, 2 * ci + 1:2 * ci + 2])
                    psum_g = pstat.tile([8, 2], dt.float32, tag="psum_g")
                    nc.tensor.matmul(psum_g[:], g_sb[:], st[:, 2 * ci:2 * ci + 2],
                                     start=True, stop=True)
                    stats8 = small.tile([8, 2], dt.float32, tag="stats8")
                    nc.scalar.activation(stats8[:], psum_g[:], AF.Copy,
                                         scale=INV_CNT)
                    m2 = small.tile([8, 1], dt.float32, tag="m2")
                    nc.vector.tensor_mul(m2[:], stats8[:, 0:1], stats8[:, 0:1])
                    var8 = small.tile([8, 1], dt.float32, tag="var8")
                    nc.vector.tensor_sub(var8[:], stats8[:, 1:2], m2[:])
                    ln8 = small.tile([8, 1], dt.float32, tag="ln8")
                    # ln(var + eps), eps via explicit bias AP
                    nc.scalar.activation(ln8[:], var8[:], AF.Ln, bias=eps8[:])
                    mr8 = small.tile([8, 2], dt.float32, tag="mr8")
                    nc.vector.tensor_copy(mr8[:, 0:1], stats8[:, 0:1])
                    # rstd = exp(-0.5*ln(var+eps)); ln/exp share a table set
                    nc.scalar.activation(mr8[:, 1:2], ln8[:], AF.Exp, scale=-0.5)
                    psum_mr = pstat.tile([128, 2], dt.float32, tag="psum_mr")
                    nc.tensor.matmul(psum_mr[:], gt_sb[:], mr8[:],
                                     start=True, stop=True)
                    mrc = small.tile([128, 2], dt.float32, tag="mrc")
                    nc.vector.tensor_copy(mrc[:], psum_mr[:])
                    nc.vector.tensor_mul(a_t[:, ci:ci + 1], mrc[:, 1:2],
                                         gam_sb[:, ci:ci + 1])
                    tmp = small.tile([128, 1], dt.float32, tag="tmpab")
                    nc.vector.tensor_mul(tmp[:], mrc[:, 0:1], a_t[:, ci:ci + 1])
                    nc.vector.tensor_sub(b_t[:, ci:ci + 1], bet_sb[:, ci:ci + 1],
                                         tmp[:])
                    nc.vector.tensor_copy(b_bf[:, ci:ci + 1], b_t[:, ci:ci + 1])

                # ---- fold GroupNorm into the projections ----------------
                # k = h@wk+bk with h = a*x+b  =>  k = x@(a*wk) + (wk^T b + bk)
                for wname, btile, b2tile in (("wk", bk_sb, bk2), ("wq", bq_sb, bq2)):
                    for co in range(CCH):
                        pb = pstat.tile([128, 1], dt.float32, tag="psum_misc",
                                        name="pb")
                        for cc in range(CCH):
                            nc.tensor.matmul(
                                pb[:],
                                w_sb[wname][:, cc * 512 + co * 128:
                                            cc * 512 + co * 128 + 128],
                                b_bf[:, cc:cc + 1],
                                start=(cc == 0), stop=(cc == CCH - 1))
                        nc.vector.tensor_add(b2tile[:, co:co + 1], pb[:],
                                             btile[:, co:co + 1])
                pbv = pstat.tile([1, 512], dt.float32, tag="psum_misc", name="pbv")
                for cc in range(CCH):
                    nc.tensor.matmul(pbv[:], b_bf[:, cc:cc + 1],
                                     w_sb["wv"][:, ts(cc, 512)],
                                     start=(cc == 0), stop=(cc == CCH - 1))
                b2v = small.tile([1, 512], dt.float32, tag="b2v")
                nc.vector.tensor_copy(b2v[:], pbv[:])
                pbb = pstat.tile([128, 512], dt.float32, tag="psum_misc", name="pbb")
                nc.tensor.matmul(pbb[:], ones1_f[:], b2v[:], start=True, stop=True)
                nc.vector.tensor_add(bvb2[:], pbb[:], bvb_sb[:])
                for wname in ("wq", "wk", "wv"):   # scale weights in place
                    for cc in range(CCH):
                        nc.vector.tensor_scalar_mul(
                            w_sb[wname][:, ts(cc, 512)],
                            w_sb[wname][:, ts(cc, 512)], a_t[:, cc:cc + 1])

                # ---- QKV ------------------------------------------------
                for co in range(CCH) if stage >= 2 else []:
                    for t in range(TT):
                        ps = pk.tile([128, 512], dt.float32, tag="pk")
                        for cc in range(CCH):
                            nc.tensor.matmul(
                                ps[:],
                                w_sb["wk"][:, cc * 512 + co * 128:
                                           cc * 512 + co * 128 + 128],
                                xt[cc][:, ts(t, 512)],
                                start=(cc == 0), stop=(cc == CCH - 1))
                        nc.scalar.activation(kt[co][:, ts(t, 512)], ps[:],
                                             AF.Identity,
                                             bias=bk2[:, co:co + 1])
                    for t in range(QB // 512):
                        ps = pk.tile([128, 512], dt.float32, tag="pk")
                        for cc in range(CCH):
                            nc.tensor.matmul(
                                ps[:],
                                w_sb["wq"][:, cc * 512 + co * 128:
                                           cc * 512 + co * 128 + 128],
                                xt[cc][:, ts(t, 512)],
                                start=(cc == 0), stop=(cc == CCH - 1))
                        nc.scalar.activation(qt[co][:, ts(t, 512)], ps[:],
                                             AF.Identity,
                                             bias=bq2[:, co:co + 1])
                for tcc in range(TC) if stage >= 2 else []:
                    ps = pk.tile([128, 512], dt.float32, tag="pk")
                    for cc in range(CCH):
                        nc.tensor.matmul(
                            ps[:],
                            xt[cc][:, ts(tcc, 128)],
                            w_sb["wv"][:, ts(cc, 512)],
                            start=(cc == 0), stop=(cc == CCH - 1))
                    nc.vector.tensor_add(v_sb[:, ts(tcc, 512)], ps[:], bvb2[:])

        # =================================================================
        # Phase C: attention panels (512 queries) + projection + residual
        # =================================================================
        with (
            tc.tile_pool(name="psp", bufs=(tune.get("psp_bufs", 2)), space="PSUM") as psp,
            tc.tile_pool(name="pop", bufs=1, space="PSUM") as pop,
            tc.tile_pool(name="ps1", bufs=1, space="PSUM") as ps1,
            tc.tile_pool(name="pexp", bufs=4) as pexp,
            tc.tile_pool(name="pot", bufs=2) as pot,
            tc.tile_pool(name="psm", bufs=2) as psm,
            tc.tile_pool(name="py", bufs=1) as py,
        ):
            if stage < 3:
                for p in range(PANELS):
                    y_sb = py.tile([128, 4 * 512], dt.float32, tag="y")
                    for qc in range(4):
                        nc.vector.tensor_copy(y_sb[:, ts(qc, 512)],
                                              xpb[:, ts(p * 4 + qc, 512)])
                    nc.sync.dma_start(
                        ap["y"].rearrange("(qc p) c -> p qc c", p=128)[
                            :, p * 4:(p + 1) * 4, :],
                        y_sb[:].rearrange("p (qc c) -> p qc c", c=512))
                return
            c_loop = tc.For_i(0, loop_c, 1) if loop_c else nullcontext()
            with c_loop:
                for p in range(PANELS):
                    qsl = ts(p, 512)
                    o_ps = [pop.tile([128, 512], dt.float32, tag=f"po{c}", name=f"po{c}")
                            for c in range(CCH)]
                    s_ps = ps1.tile([1, 512], dt.float32, tag="s_ps")

                    def emit_scores(kc, score):
                        for cc in range(CCH):
                            nc.tensor.matmul(
                                score[:], kt[cc][:, ts(kc, 128)], qt[cc][:, qsl],
                                start=(cc == 0), stop=(cc == CCH - 1))

                    depth = tune.get("c_depth", 1)
                    dummy_exp = tune.get("dummy_exp", False)
                    if dummy_exp:
                        edum = pexp.tile([128, 512], dt.bfloat16, tag="edum")
                        nc.vector.memset(edum[:], 1.0)
                    scores_q = [psp.tile([128, 512], dt.float32, tag="score",
                                         name="score")]
                    emit_scores(0, scores_q[0])
                    for kk in range(1, depth):
                        if kk < TC:
                            scores_q.append(psp.tile([128, 512], dt.float32,
                                                     tag="score", name="score"))
                            emit_scores(kk, scores_q[-1])
                    for kc in range(TC):
                        score = scores_q.pop(0)
                        if dummy_exp:
                            e = edum
                            nc.scalar.activation(
                                pexp.tile([128, 512], dt.bfloat16, tag="exp",
                                          name="exp")[:],
                                score[:], AF.Exp, scale=SCALE)
                        else:
                            e = pexp.tile([128, 512], dt.bfloat16, tag="exp")
                            nc.scalar.activation(e[:], score[:], AF.Exp, scale=SCALE)
                        if kc + depth < TC:
                            sc2 = psp.tile([128, 512], dt.float32, tag="score",
                                           name="score")
                            emit_scores(kc + depth, sc2)
                            scores_q.append(sc2)
                        for cc in range(CCH):
                            nc.tensor.matmul(
                                o_ps[cc][:],
                                v_sb[:, kc * 512 + cc * 128:
                                     kc * 512 + cc * 128 + 128],
                                e[:],
                                start=(kc == 0), stop=(kc == TC - 1))
                        nc.tensor.matmul(s_ps[:], ones_bf[:], e[:],
                                         start=(kc == 0), stop=(kc == TC - 1))

                    recip = psm.tile([1, 512], dt.float32, tag="recip")
                    nc.vector.reciprocal(recip[:], s_ps[:])
                    rb_ps = psp.tile([128, 512], dt.float32, tag="score")
                    nc.tensor.matmul(rb_ps[:], ones1_f[:], recip[:],
                                     start=True, stop=True)
                    rb = psm.tile([128, 512], dt.float32, tag="rb")
                    nc.vector.tensor_copy(rb[:], rb_ps[:])
                    ot = pot.tile([128, CCH * 512], dt.bfloat16, tag="ot")
                    for cc in range(CCH):
                        nc.vector.tensor_mul(ot[:, ts(cc, 512)], o_ps[cc][:], rb[:])
                    y_sb = py.tile([128, 4 * 512], dt.float32, tag="y")
                    for qc in range(4):
                        yp = psp.tile([128, 512], dt.float32, tag="score")
                        for cc in range(CCH):
                            nc.tensor.matmul(
                                yp[:],
                                ot[:, cc * 512 + qc * 128: cc * 512 + qc * 128 + 128],
                                w_sb["wp"][:, ts(cc, 512)],
                                start=(cc == 0), stop=(cc == CCH - 1))
                        gqc = p * 4 + qc
                        nc.vector.tensor_add(y_sb[:, ts(qc, 512)], yp[:],
                                             xpb[:, ts(gqc, 512)])
                    nc.sync.dma_start(
                        ap["y"].rearrange("(qc p) c -> p qc c", p=128)[
                            :, p * 4:(p + 1) * 4, :],
                        y_sb[:].rearrange("p (qc c) -> p qc c", c=512))


def _build(loop_ab=0, loop_c=0, stage=3, tune=None):
    import concourse.tile as tile
    from concourse import bacc, mybir

    dt = mybir.dt
    nc = bacc.Bacc("TRN2", target_bir_lowering=False, debug=False,
                   num_devices=NCORES)
    ap = {}

    def din(name, shape, dtype):
        ap[name] = nc.dram_tensor(name, list(shape), dtype,
                                  kind="ExternalInput").ap()

    din("xt", (C, N), dt.bfloat16)
    din("xblk", (QB, C), dt.float32)
    for wname in ("wq", "wk", "wv", "wp"):
        din(wname, (C, C), dt.bfloat16)
    din("bq_t", (C, 1), dt.float32)
    din("bk_t", (C, 1), dt.float32)
    din("bv_b", (128, 512), dt.float32)
    din("bp_b", (128, 512), dt.float32)
    din("gam_t", (C, 1), dt.float32)
    din("bet_t", (C, 1), dt.float32)
    din("gmat", (128, 8), dt.float32)
    din("gmat_t", (8, 128), dt.float32)
    ap["y"] = nc.dram_tensor("y", [QB, C], dt.float32, kind="ExternalOutput").ap()

    with tile.TileContext(nc) as tc:
        _emit(nc, tc, ap, loop_ab=loop_ab, loop_c=loop_c, stage=stage, tune=tune)
    nc.compile()
    return nc


def _host_inputs(x, gamma, beta, wq, bq, wk, bk, wv, bv, wp, bp):
    f32 = np.float32
    xr = np.ascontiguousarray(np.asarray(x).reshape(B, N, C), dtype=f32)
    xt_b = [np.ascontiguousarray(xr[b].T.astype(_BF16)) for b in range(B)]
    w_bf = {n: np.ascontiguousarray(np.asarray(w)).astype(_BF16)
            for n, w in (("wq", wq), ("wk", wk), ("wv", wv), ("wp", wp))}
    g = np.repeat(np.eye(8, dtype=f32), GS, axis=0)
    shared = {
        **w_bf,
        "bq_t": np.ascontiguousarray(np.asarray(bq, f32).reshape(C, 1)),
        "bk_t": np.ascontiguousarray(np.asarray(bk, f32).reshape(C, 1)),
        "bv_b": np.ascontiguousarray(
            np.broadcast_to(np.asarray(bv, f32), (128, 512))),
        "bp_b": np.ascontiguousarray(
            np.broadcast_to(np.asarray(bp, f32), (128, 512))),
        "gam_t": np.ascontiguousarray(np.asarray(gamma, f32).reshape(C, 1)),
        "bet_t": np.ascontiguousarray(np.asarray(beta, f32).reshape(C, 1)),
        "gmat": g,
        "gmat_t": np.ascontiguousarray(g.T),
    }
    in_maps = []
    for core in range(NCORES):
        b, r = divmod(core, RPB)
        qoff = r * QB
        m = dict(shared)
        # rotate tokens so this core's queries are always columns 0..QB-1
        m["xt"] = np.ascontiguousarray(
            np.concatenate([xt_b[b][:, qoff:], xt_b[b][:, :qoff]], axis=1))
        m["xblk"] = np.ascontiguousarray(xr[b, qoff:qoff + QB])
        in_maps.append(m)
    return in_maps


def kernel(x, gamma, beta, wq, bq, wk, bk, wv, bv, wp, bp):
    from concourse.bass_utils import run_bass_kernel_spmd

    if "nc" not in _BUILT:
        _BUILT["nc"] = _build()
    nc = _BUILT["nc"]
    in_maps = _host_inputs(x, gamma, beta, wq, bq, wk, bk, wv, bv, wp, bp)
    res = run_bass_kernel_spmd(nc, in_maps, list(range(NCORES)))
    out = np.empty((B, N, C), np.float32)
    for core in range(NCORES):
        b, r = divmod(core, RPB)
        out[b, r * QB:(r + 1) * QB] = res.results[core]["y"]
    return out.reshape(B, H, W, C)


# revision 20
# speedup vs baseline: 1.5913x; 1.5913x over previous
"""Self-contained Trainium2 Bass kernel for nn_AttentionBlock (B2 H64 W64 C512).

Module: GroupNorm(32 groups) -> 1x1 conv q,k,v -> full [N,N] softmax attention
        -> 1x1 proj -> residual.

Sharding: 8 cores = 2 batches x 4 query-blocks (1024 rows each).  Each core
gets its batch's full image transposed to [C, N] bf16 with the token axis
rotated so its own query block is always columns 0..1023 (attention is
permutation-invariant over keys, so the rotation needs no undo on the key
side).  GroupNorm coefficients land on the partition axis, K/V are computed
for all 4096 tokens (replicated within the 4-core batch group), attention
keeps keys on the partition axis throughout (logits here are tiny, |s|<2,
so softmax needs no max subtraction), and the fp32 residual is added from
an untransposed per-core block.
"""

import numpy as np
import ml_dtypes

B, H, W, C = 2, 64, 64, 512
N = H * W            # 4096 tokens per batch
GROUPS, GS = 32, 16
EPS = 1e-5
NCORES = 8
RPB = 4              # query row-blocks per batch
QB = N // RPB        # 1024 queries per core
CCH = C // 128       # 4 channel chunks
TT = N // 512        # 8 token tiles of 512
TC = N // 128        # 32 token chunks of 128
PANELS = QB // 512   # query panels of 512 per core
SCALE = float(C) ** -0.5
INV_CNT = 1.0 / (N * GS)

_BF16 = ml_dtypes.bfloat16
_BUILT = {}


def _emit(nc, tc, ap, loop_ab=0, loop_c=0, stage=3, tune=None):
    tune = tune or {}
    import concourse.bass as bass
    from concourse import mybir
    from contextlib import nullcontext

    dt = mybir.dt
    AF = mybir.ActivationFunctionType
    ALU = mybir.AluOpType
    AX = mybir.AxisListType
    ts = bass.ts

    with tc.tile_pool(name="persist", bufs=1) as P:
        # ---- persistent SBUF tiles ---------------------------------------
        F8 = dt.float8e4
        DR = mybir.MatmulPerfMode.DoubleRow
        # fp8 pair tiles: index i holds channel-chunk pair (2i, 2i+1) in halves
        kt8 = [P.tile([128, 2 * N], F8, tag=f"kt8{i}", name=f"kt8{i}") for i in range(2)]
        qt8 = [P.tile([128, 2 * QB], F8, tag=f"qt8{i}", name=f"qt8{i}") for i in range(2)]
        v_sb = P.tile([128, TC * 512], F8, tag="v")  # [tok%128,(tc,c)], holds 16*v
        w_sb = {}
        for wname in ("wq", "wk", "wv", "wp"):
            w_sb[wname] = P.tile([128, CCH * 512], dt.bfloat16, tag=wname, name=wname + "_sb")
            nc.sync.dma_start(w_sb[wname][:],
                              ap[wname].rearrange("(cc p) m -> p cc m", p=128))
        bq_sb = P.tile([128, CCH], dt.float32, tag="bq")
        bk_sb = P.tile([128, CCH], dt.float32, tag="bk")
        nc.sync.dma_start(bq_sb[:], ap["bq_t"].rearrange("(cc p) o -> p cc o", p=128))
        nc.sync.dma_start(bk_sb[:], ap["bk_t"].rearrange("(cc p) o -> p cc o", p=128))
        bvb_sb = P.tile([128, 512], dt.float32, tag="bvb")
        bpb_sb = P.tile([128, 512], dt.float32, tag="bpb")
        nc.sync.dma_start(bvb_sb[:], ap["bv_b"][:])
        nc.sync.dma_start(bpb_sb[:], ap["bp_b"][:])
        gam_sb = P.tile([128, CCH], dt.float32, tag="gam")
        bet_sb = P.tile([128, CCH], dt.float32, tag="bet")
        nc.sync.dma_start(gam_sb[:], ap["gam_t"].rearrange("(cc p) o -> p cc o", p=128))
        nc.sync.dma_start(bet_sb[:], ap["bet_t"].rearrange("(cc p) o -> p cc o", p=128))
        g_sb = P.tile([128, 8], dt.float32, tag="g")
        gt_sb = P.tile([8, 128], dt.float32, tag="gt")
        nc.sync.dma_start(g_sb[:], ap["gmat"][:])
        nc.sync.dma_start(gt_sb[:], ap["gmat_t"][:])
        ones8 = P.tile([128, 32], F8, tag="ones8")   # pair AP needs step%16==0
        nc.vector.memset(ones8[:], 1.0)
        ones1_f = P.tile([1, 128], dt.float32, tag="ones1_f")
        nc.vector.memset(ones1_f[:], 1.0)
        oneq_f = P.tile([1, 128], dt.float32, tag="oneq_f")
        nc.vector.memset(oneq_f[:], 1.0 / 16.0)   # folds away the v fp8 gain
        xpb = P.tile([128, RPB * 2 * 512], dt.float32, tag="xpb")
        nc.sync.dma_start(xpb[:], ap["xblk"].rearrange("(qc p) c -> p qc c", p=128))
        for gqc in range(2 * RPB):   # fold proj bias into the residual block
            nc.vector.tensor_add(xpb[:, ts(gqc, 512)], xpb[:, ts(gqc, 512)],
                                 bpb_sb[:])
        st = P.tile([128, 2 * CCH], dt.float32, tag="st")
        a_t = P.tile([128, CCH], dt.float32, tag="a_t")
        b_t = P.tile([128, CCH], dt.float32, tag="b_t")

        # =================================================================
        # Phases A+B: GroupNorm stats + normalize -> ht, then QKV.
        # =================================================================
        with (
            tc.tile_pool(name="pin", bufs=1) as pin,
            tc.tile_pool(name="small", bufs=4) as small,
        ):
            xt = [pin.tile([128, N], dt.bfloat16, tag=f"xt{i}", name=f"xt{i}") for i in range(CCH)]
            xt8 = [pin.tile([128, 2 * N], F8, tag=f"xt8{i}", name=f"xt8{i}")
                   for i in range(2)]
            for i in range(2):
                nc.sync.dma_start(
                    xt8[i][:],
                    ap["xt8"][256 * i:256 * (i + 1), :].rearrange(
                        "(h p) t -> p h t", p=128))
            wk2 = pin.tile([128, CCH * 512], F8, tag="wk2")
            wq2 = pin.tile([128, CCH * 512], F8, tag="wq2")
            wv2 = pin.tile([128, CCH * 512], F8, tag="wv2")
            a4_t = pin.tile([128, CCH], dt.float32, tag="a4_t")
            a16_t = pin.tile([128, CCH], dt.float32, tag="a16_t")
            sq_scr = pin.tile([128, N], dt.bfloat16, tag="sq_scr")
            eps8 = pin.tile([8, 1], dt.float32, tag="eps8")
            nc.vector.memset(eps8[:], EPS)
            b_bf = pin.tile([128, CCH], dt.bfloat16, tag="b_bf")
            bk2 = pin.tile([128, CCH], dt.float32, tag="bk2")
            bq2 = pin.tile([128, CCH], dt.float32, tag="bq2")
            bvb2 = pin.tile([128, 512], dt.float32, tag="bvb2")
            for ci in range(CCH):
                nc.sync.dma_start(xt[ci][:], ap["xt"][ci * 128:(ci + 1) * 128, :])

            with tc.tile_pool(name="pstat", bufs=1, space="PSUM") as pstat, \
                    tc.tile_pool(name="pk", bufs=4, space="PSUM") as pk, \
                    (tc.For_i(0, loop_ab, 1) if loop_ab else nullcontext()):
                for ci in range(CCH):
                    nc.vector.reduce_sum(st[:, 2 * ci:2 * ci + 1], xt[ci][:],
                                         axis=AX.X)
                    nc.scalar.activation(sq_scr[:], xt[ci][:], AF.Square,
                                         accum_out=st[:, 2 * ci + 1:2 * ci + 2])
                    psum_g = pstat.tile([8, 2], dt.float32, tag="psum_g")
                    nc.tensor.matmul(psum_g[:], g_sb[:], st[:, 2 * ci:2 * ci + 2],
                                     start=True, stop=True)
                    stats8 = small.tile([8, 2], dt.float32, tag="stats8")
                    nc.scalar.activation(stats8[:], psum_g[:], AF.Copy,
                                         scale=INV_CNT)
                    m2 = small.tile([8, 1], dt.float32, tag="m2")
                    nc.vector.tensor_mul(m2[:], stats8[:, 0:1], stats8[:, 0:1])
                    var8 = small.tile([8, 1], dt.float32, tag="var8")
                    nc.vector.tensor_sub(var8[:], stats8[:, 1:2], m2[:])
                    ln8 = small.tile([8, 1], dt.float32, tag="ln8")
                    # ln(var + eps), eps via explicit bias AP
                    nc.scalar.activation(ln8[:], var8[:], AF.Ln, bias=eps8[:])
                    mr8 = small.tile([8, 2], dt.float32, tag="mr8")
                    nc.vector.tensor_copy(mr8[:, 0:1], stats8[:, 0:1])
                    # rstd = exp(-0.5*ln(var+eps)); ln/exp share a table set
                    nc.scalar.activation(mr8[:, 1:2], ln8[:], AF.Exp, scale=-0.5)
                    psum_mr = pstat.tile([128, 2], dt.float32, tag="psum_mr")
                    nc.tensor.matmul(psum_mr[:], gt_sb[:], mr8[:],
                                     start=True, stop=True)
                    mrc = small.tile([128, 2], dt.float32, tag="mrc")
                    nc.vector.tensor_copy(mrc[:], psum_mr[:])
                    nc.vector.tensor_mul(a_t[:, ci:ci + 1], mrc[:, 1:2],
                                         gam_sb[:, ci:ci + 1])
                    tmp = small.tile([128, 1], dt.float32, tag="tmpab")
                    nc.vector.tensor_mul(tmp[:], mrc[:, 0:1], a_t[:, ci:ci + 1])
                    nc.vector.tensor_sub(b_t[:, ci:ci + 1], bet_sb[:, ci:ci + 1],
                                         tmp[:])
                    nc.vector.tensor_copy(b_bf[:, ci:ci + 1], b_t[:, ci:ci + 1])

                # ---- fold GroupNorm into the projections ----------------
                # k = h@wk+bk with h = a*x+b  =>  k = x@(a*wk) + (wk^T b + bk)
                for wname, btile, b2tile in (("wk", bk_sb, bk2), ("wq", bq_sb, bq2)):
                    for co in range(CCH):
                        pb = pstat.tile([128, 1], dt.float32, tag="psum_misc",
                                        name="pb")
                        for cc in range(CCH):
                            nc.tensor.matmul(
                                pb[:],
                                w_sb[wname][:, cc * 512 + co * 128:
                                            cc * 512 + co * 128 + 128],
                                b_bf[:, cc:cc + 1],
                                start=(cc == 0), stop=(cc == CCH - 1))
                        nc.vector.tensor_add(b2tile[:, co:co + 1], pb[:],
                                             btile[:, co:co + 1])
                pbv = pstat.tile([1, 512], dt.float32, tag="psum_misc", name="pbv")
                for cc in range(CCH):
                    nc.tensor.matmul(pbv[:], b_bf[:, cc:cc + 1],
                                     w_sb["wv"][:, ts(cc, 512)],
                                     start=(cc == 0), stop=(cc == CCH - 1))
                b2v = small.tile([1, 512], dt.float32, tag="b2v")
                nc.vector.tensor_copy(b2v[:], pbv[:])
                pbb = pstat.tile([128, 512], dt.float32, tag="psum_misc", name="pbb")
                nc.tensor.matmul(pbb[:], ones1_f[:], b2v[:], start=True, stop=True)
                nc.vector.tensor_add(bvb2[:], pbb[:], bvb_sb[:])
                # gains: kt8/qt8 hold 4*k, 4*q; v_sb holds 16*v (fp8 ranges)
                nc.vector.tensor_scalar_mul(bk2[:], bk2[:], 4.0)
                nc.vector.tensor_scalar_mul(bq2[:], bq2[:], 4.0)
                nc.vector.tensor_scalar_mul(bvb2[:], bvb2[:], 16.0)
                nc.vector.tensor_scalar_mul(a4_t[:], a_t[:], 4.0)
                nc.vector.tensor_scalar_mul(a16_t[:], a_t[:], 16.0)
                for wname, wdst, asrc in (("wq", wq2, a4_t), ("wk", wk2, a4_t),
                                          ("wv", wv2, a16_t)):
                    for cc in range(CCH):
                        nc.vector.tensor_scalar_mul(
                            wdst[:, ts(cc, 512)],
                            w_sb[wname][:, ts(cc, 512)], asrc[:, cc:cc + 1])

                # ---- QKV (fp8 DoubleRow: contraction pairs of c-chunks) --
                def wpair(w, i, co):
                    return w[:, i * 1024:(i + 1) * 1024].rearrange(
                        "p (h m) -> p h m", h=2)[:, :, co * 128:(co + 1) * 128]

                def wpair_full(w, i):
                    return w[:, i * 1024:(i + 1) * 1024].rearrange(
                        "p (h m) -> p h m", h=2)

                def xpair(i, lo, n):
                    return xt8[i][:].rearrange("p (h t) -> p h t", h=2)[
                        :, :, lo:lo + n]

                for co in range(CCH) if stage >= 2 else []:
                    for t in range(TT):
                        ps = pk.tile([128, 512], dt.float32, tag="pk")
                        for i in range(2):
                            nc.tensor.matmul(ps[:], wpair(wk2, i, co),
                                             xpair(i, t * 512, 512),
                                             start=(i == 0), stop=(i == 1),
                                             perf_mode=DR)
                        nc.scalar.activation(
                            kt8[co // 2][:, (co % 2) * N + t * 512:
                                         (co % 2) * N + (t + 1) * 512],
                            ps[:], AF.Identity, bias=bk2[:, co:co + 1])
                    for t in range(QB // 512):
                        ps = pk.tile([128, 512], dt.float32, tag="pk")
                        for i in range(2):
                            nc.tensor.matmul(ps[:], wpair(wq2, i, co),
                                             xpair(i, t * 512, 512),
                                             start=(i == 0), stop=(i == 1),
                                             perf_mode=DR)
                        nc.scalar.activation(
                            qt8[co // 2][:, (co % 2) * QB + t * 512:
                                         (co % 2) * QB + (t + 1) * 512],
                            ps[:], AF.Identity, bias=bq2[:, co:co + 1])
                for tcc in range(TC) if stage >= 2 else []:
                    ps = pk.tile([128, 512], dt.float32, tag="pk")
                    for i in range(2):
                        nc.tensor.matmul(ps[:], xpair(i, tcc * 128, 128),
                                         wpair_full(wv2, i),
                                         start=(i == 0), stop=(i == 1),
                                         perf_mode=DR)
                    nc.vector.tensor_add(v_sb[:, ts(tcc, 512)], ps[:], bvb2[:])

        # =================================================================
        # Phase C: attention panels (512 queries) + projection + residual
        # =================================================================
        with (
            tc.tile_pool(name="psp", bufs=(tune.get("psp_bufs", 3)), space="PSUM") as psp,
            tc.tile_pool(name="pop", bufs=1, space="PSUM") as pop,
            tc.tile_pool(name="ps1", bufs=1, space="PSUM") as ps1,
            tc.tile_pool(name="pexp", bufs=4) as pexp,
            tc.tile_pool(name="pot", bufs=2) as pot,
            tc.tile_pool(name="psm", bufs=2) as psm,
            tc.tile_pool(name="py", bufs=1) as py,
        ):
            if stage < 3:
                for p in range(PANELS):
                    y_sb = py.tile([128, 4 * 512], dt.float32, tag="y")
                    for qc in range(4):
                        nc.vector.tensor_copy(y_sb[:, ts(qc, 512)],
                                              xpb[:, ts(p * 4 + qc, 512)])
                    nc.sync.dma_start(
                        ap["y"].rearrange("(qc p) c -> p qc c", p=128)[
                            :, p * 4:(p + 1) * 4, :],
                        y_sb[:].rearrange("p (qc c) -> p qc c", c=512))
                return
            c_loop = tc.For_i(0, loop_c, 1) if loop_c else nullcontext()
            with c_loop:
                for p in range(PANELS):
                    ktv = [kt8[i][:].rearrange("p (h t) -> p h t", h=2)
                           for i in range(2)]
                    qtv = [qt8[i][:].rearrange("p (h t) -> p h t", h=2)[
                        :, :, p * 512:(p + 1) * 512] for i in range(2)]
                    o_ps = [pop.tile([128, 512], dt.float32, tag=f"po{c}", name=f"po{c}")
                            for c in range(CCH)]
                    s_ps = ps1.tile([1, 512], dt.float32, tag="s_ps")
                    onesv = ones8[:].rearrange("p (h x) -> p h x", h=2)[:, :, 0:1]

                    def emit_scores(kc, score):
                        for i in range(2):
                            nc.tensor.matmul(
                                score[:], ktv[i][:, :, kc * 128:(kc + 1) * 128],
                                qtv[i], start=(i == 0), stop=(i == 1),
                                perf_mode=DR)

                    scores_q = {}
                    for kc in range(2):
                        sc = psp.tile([128, 512], dt.float32, tag="score",
                                      name="score")
                        emit_scores(kc, sc)
                        scores_q[kc] = sc
                    for kc2 in range(TC // 2):
                        e2 = pexp.tile([128, 1024], F8, tag="exp", name="exp")
                        for h in range(2):
                            kc = 2 * kc2 + h
                            nc.scalar.activation(e2[:, h * 512:(h + 1) * 512],
                                                 scores_q.pop(kc)[:], AF.Exp,
                                                 scale=SCALE / 16.0)
                            nxt = kc + 2
                            if nxt < TC:
                                sc = psp.tile([128, 512], dt.float32,
                                              tag="score", name="score")
                                emit_scores(nxt, sc)
                                scores_q[nxt] = sc
                        e2v = e2[:].rearrange("p (h n) -> p h n", h=2)
                        vv = v_sb[:, kc2 * 1024:(kc2 + 1) * 1024].rearrange(
                            "p (h c) -> p h c", h=2)
                        for cc in range(CCH):
                            nc.tensor.matmul(
                                o_ps[cc][:], vv[:, :, cc * 128:(cc + 1) * 128],
                                e2v, start=(kc2 == 0), stop=(kc2 == TC // 2 - 1),
                                perf_mode=DR)
                        nc.tensor.matmul(s_ps[:], onesv, e2v,
                                         start=(kc2 == 0),
                                         stop=(kc2 == TC // 2 - 1), perf_mode=DR)

                    recip = psm.tile([1, 512], dt.float32, tag="recip")
                    nc.vector.reciprocal(recip[:], s_ps[:])
                    rb_ps = psp.tile([128, 512], dt.float32, tag="score")
                    nc.tensor.matmul(rb_ps[:], oneq_f[:], recip[:],
                                     start=True, stop=True)
                    rb = psm.tile([128, 512], dt.float32, tag="rb")
                    nc.vector.tensor_copy(rb[:], rb_ps[:])
                    # ot = normalized o in bf16 (rb already folds away v gain)
                    ot = pot.tile([128, CCH * 512], dt.bfloat16, tag="ot")
                    for cc in range(CCH):
                        nc.vector.tensor_mul(ot[:, ts(cc, 512)], o_ps[cc][:], rb[:])
                    y_sb = py.tile([128, 4 * 512], dt.float32, tag="y")
                    for qc in range(4):
                        yp = psp.tile([128, 512], dt.float32, tag="score")
                        for cc in range(CCH):
                            nc.tensor.matmul(
                                yp[:],
                                ot[:, cc * 512 + qc * 128: cc * 512 + qc * 128 + 128],
                                w_sb["wp"][:, ts(cc, 512)],
                                start=(cc == 0), stop=(cc == CCH - 1))
                        gqc = p * 4 + qc
                        nc.vector.tensor_add(y_sb[:, ts(qc, 512)], yp[:],
                                             xpb[:, ts(gqc, 512)])
                    nc.sync.dma_start(
                        ap["y"].rearrange("(qc p) c -> p qc c", p=128)[
                            :, p * 4:(p + 1) * 4, :],
                        y_sb[:].rearrange("p (qc c) -> p qc c", c=512))


def _build(loop_ab=0, loop_c=0, stage=3, tune=None):
    import concourse.tile as tile
    from concourse import bacc, mybir

    dt = mybir.dt
    nc = bacc.Bacc("TRN2", target_bir_lowering=False, debug=False,
                   num_devices=NCORES)
    ap = {}

    def din(name, shape, dtype):
        ap[name] = nc.dram_tensor(name, list(shape), dtype,
                                  kind="ExternalInput").ap()

    din("xt", (C, N), dt.bfloat16)
    din("xt8", (C, N), dt.float8e4)
    din("xblk", (QB, C), dt.float32)
    for wname in ("wq", "wk", "wv", "wp"):
        din(wname, (C, C), dt.bfloat16)
    din("bq_t", (C, 1), dt.float32)
    din("bk_t", (C, 1), dt.float32)
    din("bv_b", (128, 512), dt.float32)
    din("bp_b", (128, 512), dt.float32)
    din("gam_t", (C, 1), dt.float32)
    din("bet_t", (C, 1), dt.float32)
    din("gmat", (128, 8), dt.float32)
    din("gmat_t", (8, 128), dt.float32)
    ap["y"] = nc.dram_tensor("y", [QB, C], dt.float32, kind="ExternalOutput").ap()

    with tile.TileContext(nc) as tc:
        _emit(nc, tc, ap, loop_ab=loop_ab, loop_c=loop_c, stage=stage, tune=tune)
    nc.compile()
    return nc


def _host_inputs(x, gamma, beta, wq, bq, wk, bk, wv, bv, wp, bp):
    f32 = np.float32
    xr = np.ascontiguousarray(np.asarray(x).reshape(B, N, C), dtype=f32)
    xt_b = [np.ascontiguousarray(xr[b].T.astype(_BF16)) for b in range(B)]
    from concourse import mybir
    fp8 = mybir.dt.np(mybir.dt.float8e4)
    w_bf = {n: np.ascontiguousarray(np.asarray(w)).astype(_BF16)
            for n, w in (("wq", wq), ("wk", wk), ("wv", wv), ("wp", wp))}
    g = np.repeat(np.eye(8, dtype=f32), GS, axis=0)
    shared = {
        **w_bf,
        "bq_t": np.ascontiguousarray(np.asarray(bq, f32).reshape(C, 1)),
        "bk_t": np.ascontiguousarray(np.asarray(bk, f32).reshape(C, 1)),
        "bv_b": np.ascontiguousarray(
            np.broadcast_to(np.asarray(bv, f32), (128, 512))),
        "bp_b": np.ascontiguousarray(
            np.broadcast_to(np.asarray(bp, f32), (128, 512))),
        "gam_t": np.ascontiguousarray(np.asarray(gamma, f32).reshape(C, 1)),
        "bet_t": np.ascontiguousarray(np.asarray(beta, f32).reshape(C, 1)),
        "gmat": g,
        "gmat_t": np.ascontiguousarray(g.T),
    }
    in_maps = []
    for core in range(NCORES):
        b, r = divmod(core, RPB)
        qoff = r * QB
        m = dict(shared)
        # rotate tokens so this core's queries are always columns 0..QB-1
        m["xt"] = np.ascontiguousarray(
            np.concatenate([xt_b[b][:, qoff:], xt_b[b][:, :qoff]], axis=1))
        m["xt8"] = np.ascontiguousarray(m["xt"].astype(fp8))
        m["xblk"] = np.ascontiguousarray(xr[b, qoff:qoff + QB])
        in_maps.append(m)
    return in_maps


def kernel(x, gamma, beta, wq, bq, wk, bk, wv, bv, wp, bp):
    from concourse.bass_utils import run_bass_kernel_spmd

    if "nc" not in _BUILT:
        _BUILT["nc"] = _build()
    nc = _BUILT["nc"]
    in_maps = _host_inputs(x, gamma, beta, wq, bq, wk, bk, wv, bv, wp, bp)
    res = run_bass_kernel_spmd(nc, in_maps, list(range(NCORES)))
    out = np.empty((B, N, C), np.float32)
    for core in range(NCORES):
        b, r = divmod(core, RPB)
        out[b, r * QB:(r + 1) * QB] = res.results[core]["y"]
    return out.reshape(B, H, W, C)
